# revision 72
# baseline (speedup 1.0000x reference)
"""Trainium2 Bass kernel for nn_Attention_54065048322573 (v3).

XCA/Restormer-style channel attention:
  q = dwconv3x3(conv1x1(high)); k,v = split(dwconv3x3(conv1x1(low)))
  q,k L2-normalized over space; attn = softmax((q@k^T)*temp); out = conv1x1(attn@v)

Strategy: spatial sharding over H (20 rows/core + 1-row halo), all compute
local except a 139KB AllReduce of per-head gram blocks + norms.

v3 vs v2 (364us -> 250us in TimelineSim): fp8(e4m3) + DoubleRow matmuls
on the q/k path.
  - q/k 1x1 convs in fp8 DoubleRow (contraction 256 in ONE pass at 0.5
    cycles/col): inputs shipped fp8, weights [128, 2, couts] fp8
  - q,k dwconv taps: fp8 DoubleRow PAIRS of shifted windows (5 per-row
    matmuls of 2 taps each, one slot zero-padded), ALL on PE
  - v path stays bf16 end to end: quantization error on conv/tap
    inputs does NOT average away relative to the output (both signal and
    error grow as sqrt(N) in random-sign sums), so fp8 v would put ~3%
    straight into the result; q/k fp8 error only perturbs logits ~4%
    and the near-uniform softmax (|logit|<=0.05) flattens it to <0.1%
  - q,k acc stored bf16; transposes bf16; tt copies via DVE TensorCopy
    on a bf16-bitcast psum view (2x_1p mode)
  - copy-engine assignment (U/ACC/TT/OUT) and v-row splits are the main
    tuning knobs; all norms on Act (Square+accum is 1 op and k1's norm
    gates the AllReduce — keep it OFF Pool/DVE)
  - softmax emission delayed 1 stage, outmm 5 stages: keeps waiting ops
    behind ready work on the in-order Act/PE queues
  - batch-0 consts DMA interleaved with input slabs in first-use order;
    input slabs DMA'd in kc-interleaved chunks so conv 0 starts early
Runtime landmines (do NOT reintroduce without re-verifying on HW):
  - GPSIMD TensorScalarPtr / X-axis tensor_reduce / PSUM access all fail
    walrus codegen; Pool does SBUF<->SBUF TT/copy only
  - DmaTransposeAnt and InstTensorTensorReduce compile but HANG the
    axon-relay runtime; DVE pow fails the codegen ISA check
  - matmul moving-operand APs are limited to 3 dims (a 4th compiles but
    crashes execution); fp8 ifmap base addresses must be EVEN
  - fp8 PE transpose needs output element step 2 (no psum packing win)
  - DMA cannot read PSUM (bass assert)
"""

import contextlib
import functools
import os
import sys

import numpy as np

for _p in ("/opt/trn_rl_repo", os.path.expanduser("~/.axon_site/_ro/trn_rl_repo")):
    if os.path.isdir(_p) and _p not in sys.path:
        sys.path.insert(0, _p)

import ml_dtypes  # noqa: E402

FP8NP = ml_dtypes.float8_e4m3

B, C, H, W = 4, 256, 160, 160
HEADS, CH = 8, 32
NCORES = 8
RPC = H // NCORES          # rows per core = 20
WE = W + 2                 # 162 padded width
SLABR = RPC + 2            # 22 slab rows (with halo)
SRC = SLABR * WE           # 3564 slab cols
INT = RPC * W              # 3200 interior cols

TAPS = [(dy, dx) for dy in (-1, 0, 1) for dx in (-1, 0, 1)]
# DoubleRow tap pairs: slots (a, b) with positive flat-offset stride.
# (None, 8): slot0 = zero diag reading the (1,0) window (in-bounds).
PAIRS = [(0, 1), (2, 3), (4, 5), (6, 7), (None, 8)]

# v dwconv row split per v-ptile: (rows on PE bf16 diag, rows on DVE
# TSP+TT, pool-assisted rows: DVE TSP muls + Pool TT accumulate-adds)
V_SPLITS = [(12, 8, 0), (12, 8, 0)]
V_SPLITS_B3 = [(16, 4, 0), (16, 4, 0)]
# processing order of tensor-ptiles within a batch (0=q0 1=q1 2=k0 3=k1
# 4=v0 5=v1): conv(n) is emitted before taps(n-1) (software pipeline).
# q,k first: the gram->AllReduce->softmax chain closes two stages before
# batch end, overlapping the v convs/taps (critical on the last batch).
STAGE_ORDER = (0, 1, 2, 3, 4, 5)
STAGE_ORDER_LAST = (0, 1, 2, 3, 4, 5)
# norms for q0,q1,k0,k1: 'act' = Act Square+accum; 'dve' = DVE TT sq+reduce
NORM_MODE = ("act", "act", "act", "act")
# psum->sbuf copy engine assignment ('act' | 'dve')
U_COPY = ("act", "act", "act", "act", "act", "act")   # per ptile u copies
ACC_COPY = ("dve", "dve", "dve", "dve")               # q,k acc copies
TT_COPY = ("dve", "dve", "dve", "dve")                # transpose copies
OUT_COPY = ("act", "dve")                             # per-oc out copies

CONV_FLAT = [(c, min(512, SRC - c)) for c in range(0, SRC, 512)]


def _chunks(r0, nrows, step=3):
    return [(r, min(step, r0 + nrows - r)) for r in range(r0, r0 + nrows, step)]


# ---------------------------------------------------------------- host prep

def _prep_weights(q_w, q_dw_w, kv_w, kv_dw_w, proj_w, temperature):
    bf = ml_dtypes.bfloat16
    # fp8 DoubleRow conv weights: [128, 2, couts], w8[p, i, oc] = W[oc, i*128+p]
    wq8 = np.ascontiguousarray(
        q_w[:, :, 0, 0].T.reshape(2, 128, 256).transpose(1, 0, 2), dtype=FP8NP)
    wkv8 = np.ascontiguousarray(
        kv_w[:, :, 0, 0].T.reshape(2, 128, 512).transpose(1, 0, 2),
        dtype=FP8NP)
    wkvb = np.ascontiguousarray(
        kv_w[:, :, 0, 0].T.reshape(2, 128, 512), dtype=bf)
    wproj = np.ascontiguousarray(
        proj_w[:, :, 0, 0].T.reshape(2, 128, 256), dtype=bf)

    dwq = np.asarray(q_dw_w, np.float32)[:, 0].reshape(C, 9)
    dwkv = np.asarray(kv_dw_w, np.float32)[:, 0].reshape(2 * C, 9)
    dwqk = np.concatenate([dwq, dwkv[:C]], axis=0)        # [512, 9] q,k taps
    dwv = dwkv[C:]                                        # [256, 9] v taps

    # fp8 paired diagonals for q,k taps: [128, 4, 5, 2, 128]
    dwp8 = np.zeros((128, 4, 5, 2, 128), dtype=FP8NP)
    for pt in range(4):
        for j, (ta, tb) in enumerate(PAIRS):
            for slot, t in ((0, ta), (1, tb)):
                if t is not None:
                    np.fill_diagonal(dwp8[:, pt, j, slot, :],
                                     dwqk[pt * 128:(pt + 1) * 128, t])
    dwp8 = np.ascontiguousarray(dwp8.reshape(128, 4 * 5 * 2 * 128))

    # bf16 diagonals for v PE taps: [128, 9, 2, 128]
    dwdv = np.zeros((128, 9, 2, 128), dtype=bf)
    for t in range(9):
        for vp in range(2):
            np.fill_diagonal(dwdv[:, t, vp, :],
                             dwv[vp * 128:(vp + 1) * 128, t])
    dwdv = np.ascontiguousarray(dwdv.reshape(128, 9 * 2 * 128))

    # f32 tap vectors for v DVE taps: [128, 2, 9]
    dwvec = np.ascontiguousarray(
        dwv.reshape(2, 128, 9).transpose(1, 0, 2), dtype=np.float32)

    tmpr = np.repeat(np.asarray(temperature, np.float32).reshape(HEADS), CH)
    tmpr = np.ascontiguousarray(tmpr.reshape(2, 128).T)      # [128, 2]
    ident = np.ascontiguousarray(np.eye(128, dtype=bf))
    return dict(wq8=wq8, wkv8=wkv8, wkvb=wkvb, wproj=wproj, dwp8=dwp8,
                dwdv=dwdv, dwvec=dwvec, tmpr=tmpr, ident=ident)


def _prep_slabs(low, high):
    """Per-core input slabs [B, 2, 128, SRC], zero halo/pad.

    high/low shipped fp8 for the q/k convs; low ALSO shipped bf16 for the
    v conv (fp8 conv error lands directly in the output — v stays bf16).
    """
    out = {}
    for name, x, dt in (("high_s", high, FP8NP), ("low_s", low, FP8NP),
                        ("low_b", low, ml_dtypes.bfloat16)):
        xp = np.zeros((B, C, H + 2, WE), dtype=dt)
        xp[:, :, 1:-1, 1:W + 1] = x
        out[name] = [
            np.ascontiguousarray(
                xp[:, :, RPC * i: RPC * i + SLABR, :].reshape(
                    B, 2, 128, SRC))
            for i in range(NCORES)
        ]
    return out


# ---------------------------------------------------------------- device build

@functools.lru_cache(maxsize=6)
def _build_nc(use_collective=True, loop_n=0):
    import concourse.bass as bass
    import concourse.mybir as mybir
    import concourse.tile as tile
    from concourse import bacc

    f32, bf16 = mybir.dt.float32, mybir.dt.bfloat16
    fp8 = mybir.dt.float8e4
    AOP = mybir.AluOpType
    AF = mybir.ActivationFunctionType
    AX = mybir.AxisListType
    PM = mybir.MatmulPerfMode

    nc = bacc.Bacc("TRN2", target_bir_lowering=False, debug=False,
                   num_devices=NCORES)

    hi_d = nc.dram_tensor("high_s", [B, 2, 128, SRC], fp8,
                          kind="ExternalInput").ap()
    lo_d = nc.dram_tensor("low_s", [B, 2, 128, SRC], fp8,
                          kind="ExternalInput").ap()
    lob_d = nc.dram_tensor("low_b", [B, 2, 128, SRC], bf16,
                           kind="ExternalInput").ap()
    wkvb_d = nc.dram_tensor("wkvb", [2, 128, 512], bf16,
                            kind="ExternalInput").ap()
    wq8_d = nc.dram_tensor("wq8", [128, 2, 256], fp8,
                           kind="ExternalInput").ap()
    wkv8_d = nc.dram_tensor("wkv8", [128, 2, 512], fp8,
                            kind="ExternalInput").ap()
    wproj_d = nc.dram_tensor("wproj", [2, 128, 256], bf16,
                             kind="ExternalInput").ap()
    dwp8_d = nc.dram_tensor("dwp8", [128, 4 * 5 * 2 * 128], fp8,
                            kind="ExternalInput").ap()
    dwdv_d = nc.dram_tensor("dwdv", [128, 9 * 2 * 128], bf16,
                            kind="ExternalInput").ap()
    dwvec_d = nc.dram_tensor("dwvec", [128, 2, 9], f32,
                             kind="ExternalInput").ap()
    tmpr_d = nc.dram_tensor("tmpr", [128, 2], f32, kind="ExternalInput").ap()
    ident_d = nc.dram_tensor("ident", [128, 128], bf16,
                             kind="ExternalInput").ap()
    out_d = nc.dram_tensor("out", [B, 2, 128, INT], bf16,
                           kind="ExternalOutput").ap()
    cc_in = nc.dram_tensor("cc_in", [B, 2, 128, 34], f32).ap()
    cc_out = nc.dram_tensor("cc_out", [B, 2, 128, 34], f32,
                            addr_space="Shared").ap()
    rk_dram = nc.dram_tensor("rk_tmp", [B, 256], f32).ap()

    with tile.TileContext(nc) as tc, contextlib.ExitStack() as ctx:
        ec = ctx.enter_context
        if loop_n:
            ec(tc.For_i(0, loop_n, 1))
        consts = ec(tc.tile_pool(name="consts", bufs=1))
        inp_p = ec(tc.tile_pool(name="inp", bufs=1))
        u_p = ec(tc.tile_pool(name="u", bufs=2))
        acc_p = ec(tc.tile_pool(name="acc", bufs=1))
        vt_p = ec(tc.tile_pool(name="vt", bufs=2))
        tt_p = ec(tc.tile_pool(name="tt", bufs=1))
        sq_p = ec(tc.tile_pool(name="sqp", bufs=1))
        dvt_p = ec(tc.tile_pool(name="dvt", bufs=2))
        small_p = ec(tc.tile_pool(name="small", bufs=2))
        osb_p = ec(tc.tile_pool(name="osb", bufs=2))
        att_p = ec(tc.tile_pool(name="att", bufs=2))
        mbt_p = ec(tc.tile_pool(name="mbt", bufs=2))

        cv_ps = ec(tc.tile_pool(name="cvps", bufs=2, space="PSUM"))
        dw_ps = ec(tc.tile_pool(name="dwps", bufs=2, space="PSUM"))
        gm_ps = ec(tc.tile_pool(name="gmps", bufs=1, space="PSUM"))
        mm_ps = ec(tc.tile_pool(name="mmps", bufs=2, space="PSUM"))

        # ---- constants
        wq8_sb = consts.tile([128, 2, 256], fp8, tag="wq8", name="wq8")
        wkv8_sb = consts.tile([128, 2, 512], fp8, tag="wkv8", name="wkv8")
        wkvb_sb = [consts.tile([128, 512], bf16, tag=f"wkvb{k}",
                               name=f"wkvb{k}") for k in range(2)]
        wproj_sb = [consts.tile([128, 256], bf16, tag=f"wp{k}", name=f"wp{k}")
                    for k in range(2)]
        nc.sync.dma_start(out=wq8_sb[:], in_=wq8_d)
        nc.sync.dma_start(out=wkv8_sb[:], in_=wkv8_d)
        for k in range(2):
            nc.sync.dma_start(out=wkvb_sb[k][:], in_=wkvb_d[k])
        tmpr_sb = consts.tile([128, 2], f32, tag="tmpr", name="tmpr")
        nc.sync.dma_start(out=tmpr_sb[:], in_=tmpr_d)
        # tap/transpose consts are deferred behind batch 0's input slabs
        # (first needed one conv later); conv weights stay upfront
        dwp8_sb = consts.tile([128, 4 * 5 * 2 * 128], fp8, tag="dwp8",
                              name="dwp8")
        dwdv_sb = consts.tile([128, 9 * 2 * 128], bf16, tag="dwdv",
                              name="dwdv")
        dwvec_sb = consts.tile([128, 2, 9], f32, tag="dwvec", name="dwvec")
        ident_sb = consts.tile([128, 128], bf16, tag="ident", name="ident")

        def dwpair(pt, j):
            # [128, 2, 128] fp8 lhsT for tap pair j of q/k ptile pt
            base = dwp8_sb[:]
            off = base.offset + (pt * 5 + j) * 2 * 128
            return bass.AP(tensor=base.tensor, offset=off,
                           ap=[list(base.ap[0]), [128, 2], [1, 128]])

        def dwdiag_v(t, vp):
            i = t * 2 + vp
            return dwdv_sb[:, i * 128:(i + 1) * 128]

        def uwin(u, dy, dx, r0, nr):
            # [128, nr, W] window of flat u slab for tap (dy, dx), rows r0..
            uap = u[:]
            base = uap.offset + (r0 + 1 + dy) * WE + (1 + dx)
            return bass.AP(tensor=uap.tensor, offset=base,
                           ap=[list(uap.ap[0]), [WE, nr], [1, W]])

        def pair_rhs(u, j, r):
            # [128, 2, W] pair of shifted windows for output row r.
            # NOTE: moving-operand APs are limited to 3 dims on this runtime
            # (a 4th dim compiles but crashes execution) — hence per-row.
            ta, tb = PAIRS[j]
            uap = u[:]
            if ta is None:
                # zero-diag slot anchored at col 0 of row r+2 (even offset —
                # odd fp8 ifmap base addresses crash the runtime), stride 2
                # reaches tap 8's (dy=1, dx=1) window in slot 1.
                base = uap.offset + (r + 2) * WE
                stride = 2
            else:
                dy0, dx0 = TAPS[ta]
                dyb, dxb = TAPS[tb]
                stride = (dyb * WE + dxb) - (dy0 * WE + dx0)
                base = uap.offset + (r + 1 + dy0) * WE + (1 + dx0)
            return bass.AP(tensor=uap.tensor, offset=base,
                           ap=[list(uap.ap[0]), [stride, 2], [1, W]])

        vt_tiles = {}
        mbt_tiles = {}

        # ================= phase 3: softmax + fused attn-proj =================
        def emit_softmax(b):
            red2 = att_p.tile([128, 2, 34], f32, tag="red2", name="red2")
            nc.sync.dma_start(out=red2[:],
                              in_=cc_out[b].rearrange("g p e -> p g e"))
            red = {g: red2[:, g, :] for g in range(2)}

            qn = att_p.tile([128, 2], f32, tag="qn", name="qn")
            nc.scalar.activation(out=qn[:], in_=red2[:, :, 32], func=AF.Sqrt)
            rqr = att_p.tile([128, 2], f32, tag="rqr", name="rqr")
            nc.vector.reciprocal(rqr[:], qn[:])
            rqt = att_p.tile([128, 2], f32, tag="rqt", name="rqt")
            nc.vector.tensor_tensor(out=rqt[:], in0=rqr[:], in1=tmpr_sb[:],
                                    op=AOP.mult)
            # k norms fetched one-per-partition [128, 2] (wide ops, and the
            # scatter-back writes the same rk_dram row layout)
            knrow = att_p.tile([128, 2], f32, tag="knrow", name="knrow")
            base = cc_out[b]
            kn_part = bass.AP(tensor=base.tensor, offset=base.offset + 33,
                              ap=[[34, 128], [34 * 128, 2]])
            nc.sync.dma_start(out=knrow[:], in_=kn_part)
            krt = att_p.tile([128, 2], f32, tag="krt", name="krt")
            nc.scalar.activation(out=krt[:], in_=knrow[:], func=AF.Sqrt)
            rkrow = att_p.tile([128, 2], f32, tag="rkrow", name="rkrow")
            nc.vector.reciprocal(rkrow[:], krt[:])
            rkd = rk_dram[b]
            rk_scat = bass.AP(tensor=rkd.tensor, offset=rkd.offset,
                              ap=[[1, 128], [128, 2]])
            nc.sync.dma_start(out=rk_scat, in_=rkrow[:])
            rkmat = att_p.tile([128, 256], f32, tag="rkmat", name="rkmat")
            rb = rk_dram[b]
            bcast = bass.AP(tensor=rb.tensor, offset=rb.offset,
                            ap=[[0, 128], [1, 256]])
            nc.sync.dma_start(out=rkmat[:], in_=bcast)

            # logits * softmax without max-subtraction: |logit| <= temp
            lg = att_p.tile([128, 2, 32], f32, tag="lg", name="lg")
            for g in range(2):
                for h in range(4):
                    sl = slice(h * 32, (h + 1) * 32)
                    nc.vector.scalar_tensor_tensor(
                        out=lg[sl, g, :], in0=red[g][sl, 0:32],
                        scalar=rqt[sl, g:g + 1],
                        in1=rkmat[sl, g * 128 + h * 32:g * 128 + (h + 1) * 32],
                        op0=AOP.mult, op1=AOP.mult)
            ee = att_p.tile([128, 2, 32], bf16, tag="ee", name="ee")
            nc.scalar.activation(out=ee[:], in_=lg[:], func=AF.Exp)
            ssum = att_p.tile([128, 2], f32, tag="ssum", name="ssum")
            nc.vector.reduce_sum(ssum[:], ee[:], axis=AX.X)
            rs = att_p.tile([128, 2], f32, tag="rs", name="rs")
            nc.vector.reciprocal(rs[:], ssum[:])
            rsb = bass.AP(tensor=rs.tensor, offset=rs.offset,
                          ap=[list(rs.ap[0]), [1, 2], [0, 32]])
            nc.vector.tensor_tensor(out=ee[:], in0=ee[:], in1=rsb,
                                    op=AOP.mult)

            # abdT[g][d, c] = A[c, d] (block-diag A^T), bf16
            for g in range(2):
                abdT = att_p.tile([128, 128], bf16, tag=f"abdT{g}",
                                  name=f"abdT{g}")
                nc.vector.memset(abdT[:], 0.0)
                for h in range(4):
                    sl = slice(h * 32, (h + 1) * 32)
                    nc.vector.transpose(out=abdT[sl, sl], in_=ee[sl, g, :])
                ps = mm_ps.tile([128, 512], f32, tag="mm", name="mbps")
                nc.tensor.matmul(ps[:, 0:256], lhsT=abdT[:],
                                 rhs=wproj_sb[g][:], start=True, stop=True)
                mbt = mbt_p.tile([128, 256], bf16, tag=f"mbt{g}",
                                 name=f"mbt{g}_{b}")
                mbt_tiles[(b, g)] = mbt
                nc.scalar.copy(out=mbt[:], in_=ps[:, 0:256])

        def emit_outmm(b):
            mbt = {g: mbt_tiles[(b, g)] for g in range(2)}
            last = b == B - 1
            for oc in range(2):
                obs = osb_p.tile([128, INT], bf16, tag=f"ob{oc}",
                                 name=f"ob{oc}")
                for ci, c0 in enumerate(range(0, INT, 512)):
                    cw = min(512, INT - c0)
                    ps = mm_ps.tile([128, 512], f32, tag="mm", name="mm")
                    for g in range(2):
                        nc.tensor.matmul(
                            ps[:, :cw],
                            lhsT=mbt[g][:, oc * 128:(oc + 1) * 128],
                            rhs=vt_tiles[(b, g)][:, c0:c0 + cw],
                            start=(g == 0), stop=(g == 1))
                    # last batch: alternate copy engines + per-chunk DMA so
                    # the drain tail is halved and overlaps the output DMA
                    eng = ("act" if ci % 2 else "dve") if last \
                        else OUT_COPY[oc]
                    psum_copy(eng, obs[:, c0:c0 + cw], ps[:, :cw])
                    if last:
                        nc.sync.dma_start(out=out_d[b, oc, :, c0:c0 + cw],
                                          in_=obs[:, c0:c0 + cw])
                if not last:
                    nc.sync.dma_start(out=out_d[b, oc], in_=obs[:])

        # =================== phase 1: conv + dwconv + gram ===================
        ORDER = (("q0", "hi", 0), ("q1", "hi", 1), ("k0", "lo", 0),
                 ("k1", "lo", 1), ("v0", "lo", 2), ("v1", "lo", 3))
        ins = {}
        nrm = {}
        tts = {}

        def copy_eng(which):
            return nc.vector.tensor_copy if which == "dve" else None

        def psum_copy(which, out, in_):
            if which == "dve":
                nc.vector.tensor_copy(out, in_)
            else:
                nc.scalar.copy(out=out, in_=in_)

        def emit_conv(b, ptg):
            nm, src, o = ORDER[ptg]
            if ptg >= 4:
                # v conv: bf16 2-pass (fp8 conv error lands in the output)
                u = u_p.tile([128, SRC], bf16, tag=f"uv{ptg}",
                             name=f"u{nm}_{b}", bufs=1)
                inv = ins[(b, "lob")]
                for c0, cw in CONV_FLAT:
                    ps = cv_ps.tile([128, 512], f32, tag="cv", name="cv")
                    for kc in range(2):
                        nc.tensor.matmul(
                            ps[:, :cw],
                            lhsT=wkvb_sb[kc][:, o * 128:(o + 1) * 128],
                            rhs=inv[:, kc, c0:c0 + cw],
                            start=(kc == 0), stop=(kc == 1))
                    psum_copy(U_COPY[ptg], u[:, c0:c0 + cw], ps[:, :cw])
                return u
            wsb = wq8_sb if src == "hi" else wkv8_sb
            inv = ins[(b, src)]
            u = u_p.tile([128, SRC], fp8, tag="u", name=f"u{nm}_{b}")
            for c0, cw in CONV_FLAT:
                ps = cv_ps.tile([128, 512], f32, tag="cv", name="cv")
                nc.tensor.matmul(
                    ps[:, :cw], lhsT=wsb[:, :, o * 128:(o + 1) * 128],
                    rhs=inv[:, :, c0:c0 + cw],
                    start=True, stop=True, perf_mode=PM.DoubleRow)
                psum_copy(U_COPY[ptg], u[:, c0:c0 + cw], ps[:, :cw])
            return u

        def emit_taps(b, ptg, u):
            nm = ORDER[ptg][0]
            if ptg >= 4:
                emit_taps_v(b, ptg, u)
                return
            acc = acc_p.tile([128, INT], bf16, tag=f"a{ptg}",
                             name=f"a{ptg}_{b}")
            accv = acc[:].rearrange("p (r w) -> p r w", r=RPC)
            for r0, nr in _chunks(0, RPC):
                ps = dw_ps.tile([128, 3, W], f32, tag="dw", name="dw")
                for ri in range(nr):
                    for j in range(5):
                        nc.tensor.matmul(
                            ps[:, ri, :], lhsT=dwpair(ptg, j),
                            rhs=pair_rhs(u, j, r0 + ri),
                            start=(j == 0), stop=(j == 4),
                            perf_mode=PM.DoubleRow)
                psum_copy(ACC_COPY[ptg], accv[:, r0:r0 + nr, :],
                          ps[:, :nr, :])

            # norms
            nt = small_p.tile([128, 1], f32, tag=f"nb{ptg}", name=f"nb{ptg}")
            nrm[(b, ptg)] = nt
            if NORM_MODE[ptg] == "dve":
                scr = sq_p.tile([128, INT], bf16, tag="sqd", name="sq")
                nc.vector.tensor_tensor(out=scr[:], in0=acc[:], in1=acc[:],
                                        op=AOP.mult)
                nc.vector.tensor_reduce(out=nt[:], in_=scr[:], axis=AX.X,
                                        op=AOP.add)
            elif NORM_MODE[ptg] == "pool":
                # square on the (otherwise idle) Pool engine, reduce on DVE
                scr = sq_p.tile([128, INT], bf16, tag="sqp", name="sq")
                nc.gpsimd.tensor_tensor(out=scr[:], in0=acc[:], in1=acc[:],
                                        op=AOP.mult)
                nc.vector.tensor_reduce(out=nt[:], in_=scr[:], axis=AX.X,
                                        op=AOP.add)
            else:
                scr = sq_p.tile([128, INT], bf16, tag="sqa", name="sq")
                nc.scalar.activation(out=scr[:], in_=acc[:], func=AF.Square,
                                     accum_out=nt[:])

            # transposes: bf16 via PE into psum, copied out in groups of 8
            # (each transpose = 128 bf16 = 64 f32 slots; 8 per 512-f32 bank)
            tt = tt_p.tile([128, 25, 128], bf16, tag=f"tt{ptg}",
                           name=f"tt{ptg}")
            tts[(b, ptg)] = tt
            for j0 in range(0, 25, 8):
                cnt = min(8, 25 - j0)
                ps = mm_ps.tile([128, 512], f32, tag="mm", name="tr")
                for j in range(cnt):
                    pv = ps[:, j * 64:(j + 1) * 64].bitcast(bf16)
                    c0 = (j0 + j) * 128
                    nc.tensor.transpose(pv, in_=acc[:, c0:c0 + 128],
                                        identity=ident_sb[:])
                src_ap = ps[:, 0:cnt * 64].bitcast(bf16)
                dst_ap = tt[:, j0:j0 + cnt, :].bitcast(bf16)
                if TT_COPY[ptg] == "dve":
                    nc.vector.tensor_copy(dst_ap, src_ap)
                else:
                    nc.scalar.copy(
                        out=tt[:, j0:j0 + cnt, :].bitcast(mybir.dt.uint16),
                        in_=ps[:, 0:cnt * 64].bitcast(mybir.dt.uint16))

        def emit_taps_v(b, ptg, u):
            vp = ptg - 4
            acc = vt_p.tile([128, INT], bf16, tag=f"vt{vp}",
                            name=f"vt{b}_{vp}")
            vt_tiles[(b, vp)] = acc
            accv = acc[:].rearrange("p (r w) -> p r w", r=RPC)

            splits = V_SPLITS_B3 if b == B - 1 else V_SPLITS
            r_pe, r_dve, r_pa = splits[vp]

            for r0, nr in _chunks(0, r_pe):
                ps = dw_ps.tile([128, 3, W], f32, tag="dw", name="dw")
                for t in range(9):
                    nc.tensor.matmul(
                        ps[:, :nr, :], lhsT=dwdiag_v(t, vp),
                        rhs=uwin(u, *TAPS[t], r0, nr),
                        start=(t == 0), stop=(t == 8))
                nc.scalar.copy(out=accv[:, r0:r0 + nr, :], in_=ps[:, :nr, :])

            def wv(t):
                return dwvec_sb[:, vp, t:t + 1]

            if r_dve:
                dv = accv[:, r_pe:r_pe + r_dve, :]
                nc.vector.tensor_scalar_mul(
                    dv, uwin(u, *TAPS[0], r_pe, r_dve), wv(0))
                for t in range(1, 9):
                    tmp = dvt_p.tile([128, 10, W], bf16, tag="dvt",
                                     name="dvt")
                    nc.vector.tensor_scalar_mul(
                        tmp[:, :r_dve, :], uwin(u, *TAPS[t], r_pe, r_dve),
                        wv(t))
                    nc.vector.tensor_tensor(out=dv, in0=dv,
                                            in1=tmp[:, :r_dve, :],
                                            op=AOP.add)

            if r_pa:
                pa0 = r_pe + r_dve
                pv = accv[:, pa0:pa0 + r_pa, :]
                nc.vector.tensor_scalar_mul(pv, uwin(u, *TAPS[0], pa0, r_pa),
                                            wv(0))
                for t in range(1, 9):
                    tmp = dvt_p.tile([128, 4, W], bf16, tag="pat",
                                     name="pat", bufs=2)
                    nc.vector.tensor_scalar_mul(
                        tmp[:, :r_pa, :], uwin(u, *TAPS[t], pa0, r_pa), wv(t))
                    nc.gpsimd.tensor_tensor(out=pv, in0=pv,
                                            in1=tmp[:, :r_pa, :],
                                            op=AOP.add)

        def emit_gram_stage(b):
            gram = {}
            for g in range(2):
                gp = gm_ps.tile([128, 128], f32, tag=f"g{g}", name=f"g{g}")
                gram[g] = gp
                for ci in range(25):
                    nc.tensor.matmul(gp[:], lhsT=tts[(b, g)][:, ci, :],
                                     rhs=tts[(b, 2 + g)][:, ci, :],
                                     start=(ci == 0), stop=(ci == 24))
            for g in range(2):
                sg = small_p.tile([128, 34], f32, tag="stage", name="stage")
                for h in range(4):
                    sl = slice(h * 32, (h + 1) * 32)
                    nc.scalar.copy(out=sg[sl, 0:32], in_=gram[g][sl, sl])
                nc.vector.tensor_copy(sg[:, 32:33], nrm[(b, g)][:])
                nc.vector.tensor_copy(sg[:, 33:34], nrm[(b, 2 + g)][:])
                nc.sync.dma_start(out=cc_in[b, g], in_=sg[:])

        def emit_cc(b):
            if use_collective:
                nc.gpsimd.collective_compute(
                    "AllReduce", mybir.AluOpType.add,
                    replica_groups=[list(range(NCORES))],
                    ins=[cc_in[b]], outs=[cc_out[b]])
            else:  # TimelineSim profiling build (no collectives allowed)
                nc.sync.dma_start(out=cc_out[b], in_=cc_in[b])

        pend = None
        outmm_due = []
        sm_due = []

        def tick_due():
            # softmax of batch b is emitted one stage after its AllReduce so
            # its Act ops (waiting on the cc round-trip) queue BEHIND the
            # next stage's copy work on Act's in-order queue.
            for e in sm_due:
                e[1] -= 1
            while sm_due and sm_due[0][1] <= 0:
                emit_softmax(sm_due.pop(0)[0])
            for e in outmm_due:
                e[1] -= 1
            while outmm_due and outmm_due[0][1] <= 0:
                emit_outmm(outmm_due.pop(0)[0])

        for b in range(B):
            order = STAGE_ORDER if b < B - 1 else STAGE_ORDER_LAST
            for si, ptg in enumerate(order):
                if si == 0:
                    # chunked input DMAs, kc-interleaved, so the first conv
                    # chunk can start before the whole slab lands; batch 0
                    # slots the deferred consts in first-use order between
                    # the input tensors
                    for nm, src_d, dt, nch in (("hi", hi_d, fp8, 4),
                                               ("lo", lo_d, fp8, 2),
                                               ("lob", lob_d, bf16, 4)):
                        ti = inp_p.tile([128, 2, SRC], dt,
                                        tag=f"in_{nm}", name=f"in_{nm}{b}")
                        step = -(-SRC // nch)
                        for c0 in range(0, SRC, step):
                            cw = min(step, SRC - c0)
                            for kc in range(2):
                                nc.sync.dma_start(
                                    out=ti[:, kc, c0:c0 + cw],
                                    in_=src_d[b, kc, :, c0:c0 + cw])
                        ins[(b, nm)] = ti
                        if b == 0 and nm == "hi":
                            nc.sync.dma_start(out=dwp8_sb[:], in_=dwp8_d)
                            nc.sync.dma_start(out=ident_sb[:], in_=ident_d)
                        if b == 0 and nm == "lo":
                            nc.sync.dma_start(out=dwdv_sb[:], in_=dwdv_d)
                            nc.sync.dma_start(out=dwvec_sb[:], in_=dwvec_d)
                    if b == 0:
                        for k in range(2):
                            nc.sync.dma_start(out=wproj_sb[k][:],
                                              in_=wproj_d[k])
                u = emit_conv(b, ptg)
                if pend is not None:
                    pb, pptg, pu = pend
                    emit_taps(pb, pptg, pu)
                    if pptg == 3:  # k1 done: close out batch pb
                        emit_gram_stage(pb)
                        emit_cc(pb)
                        sm_due.append([pb, 1])
                        outmm_due.append([pb, 5])
                tick_due()
                pend = (b, ptg, u)
        pb, pptg, pu = pend
        emit_taps(pb, pptg, pu)
        while sm_due:
            emit_softmax(sm_due.pop(0)[0])
        while outmm_due:
            emit_outmm(outmm_due.pop(0)[0])

    nc.compile()
    return nc


# ---------------------------------------------------------------- run

def _make_in_maps(inputs):
    low = np.asarray(inputs["low"], dtype=np.float32)
    high = np.asarray(inputs["high"], dtype=np.float32)
    wd = _prep_weights(np.asarray(inputs["q_w"]), np.asarray(inputs["q_dw_w"]),
                       np.asarray(inputs["kv_w"]),
                       np.asarray(inputs["kv_dw_w"]),
                       np.asarray(inputs["proj_w"]),
                       np.asarray(inputs["temperature"]))
    slabs = _prep_slabs(low, high)
    in_maps = []
    for i in range(NCORES):
        m = dict(wd)
        m["high_s"] = slabs["high_s"][i]
        m["low_s"] = slabs["low_s"][i]
        m["low_b"] = slabs["low_b"][i]
        in_maps.append(m)
    return in_maps


def _run(inputs, trace=False):
    from concourse.bass_utils import run_bass_kernel_spmd

    in_maps = _make_in_maps(inputs)
    nc = _build_nc()
    res = run_bass_kernel_spmd(nc, in_maps, list(range(NCORES)), trace=trace)

    out = np.empty((B, C, H, W), dtype=np.float32)
    for i in range(NCORES):
        o = np.asarray(res.results[i]["out"]).reshape(B, C, RPC, W)
        out[:, :, RPC * i:RPC * (i + 1), :] = o.astype(np.float32)
    return out, res


def kernel(**inputs):
    out, _ = _run(inputs, trace=False)
    return out


def build_timing(inputs, loop_n=1):
    """in_maps + nc for the timing build (For_i loop, no collectives)."""
    in_maps = _make_in_maps(inputs)
    nc = _build_nc(use_collective=False, loop_n=loop_n)
    return in_maps, nc
